# revision 14
# baseline (speedup 1.0000x reference)
"""GATv2 policy network on 8 Trainium2 NeuronCores.

Row-parallel attention: each core owns 128 source rows i of the N=1024
graph. The O(N^2*H*D) pair tensor is never materialized in HBM: per
local row i we build T_i = relu(wrhT + wlh_i) (the relu part of
leaky_relu = 0.2*x + 0.8*relu(x)) in SBUF and contract it against the
attention vector on the PE with T_i as the stationary operand, so the
logits land already transposed [j, (i,h)] filling all 8 PSUM banks.
The 0.2-linear term in wrh is pre-accumulated into PSUM by a broadcast
matmul; the wlh linear term is constant over j and cancels in softmax.
h is all-gathered between layers; a [1,128] all-reduce feeds the value
head's mean-pool.
"""

import numpy as np
import ml_dtypes
from contextlib import ExitStack

import concourse.bass as bass
import concourse.tile as tile
from concourse import bacc, mybir

f32 = mybir.dt.float32
f16 = mybir.dt.float16
bf16 = mybir.dt.bfloat16
f32r = mybir.dt.float32r
AF = mybir.ActivationFunctionType
ALU = mybir.AluOpType
AX = mybir.AxisListType

N = 1024
HID = 128
NLAYERS = 3
HEADS = 4
HD = 32
NCORES = 8
NL = N // NCORES  # 128 local rows
EPS = 1e-5

# stage-B T-build engine split across DVE / ScalarE / GpSimd,
# shares ~ inverse of per-inst cost (327 / 1146 / 1517 ns)
_PAT = ['D', 'D', 'A', 'D', 'D', 'D', 'D', 'A', 'D', 'D', 'D', 'A', 'D', 'D', 'D']
_ENG = [_PAT[i % 15] for i in range(NL)]


def build(enable_asserts=False, collectives=True, reps=1):
    import contextlib
    nc = bacc.Bacc("TRN2", target_bir_lowering=False, debug=False,
                   enable_asserts=enable_asserts, num_devices=NCORES)

    di = {}

    def inp(name, shape, dt=f32):
        di[name] = nc.dram_tensor(name, list(shape), dt, kind="ExternalInput")
        return di[name]

    inp("nfT", [3, N]); inp("nfT_sh", [3, NL]); inp("adjT_sh", [N, NL])
    inp("Wp", [3, HID])
    for l in range(NLAYERS):
        inp(f"Wl{l}", [HID, HID]); inp(f"Wr{l}", [HID, HID]); inp(f"Wv{l}", [HID, HID])
        inp(f"a02_{l}", [HID, HEADS], f16); inp(f"a08_{l}", [HID, HEADS], f16)
        inp(f"gam{l}", [1, HID]); inp(f"bet{l}", [1, HID])
    inp("ones1", [1, HID]); inp("onesb", [HID, 1], bf16); inp("eye", [HID, HID])
    inp("bp_c", [HID, 1]); inp("bn1_c", [64, 1]); inp("bn2_c", [1, 1])
    inp("bd1_c", [64, 1]); inp("bd2_c", [2, 1]); inp("bv1_c", [64, 1]); inp("bv2_c", [1, 1])
    inp("Wn1", [HID, 64]); inp("Wn2", [64, 1])
    inp("Wd1", [HID, 64]); inp("Wd2", [64, 2])
    inp("Wu1", [HID, 64]); inp("Wu2", [64, 1])

    logits_o = nc.dram_tensor("logits_o", [1, NL], f32, kind="ExternalOutput")
    dmu_o = nc.dram_tensor("dmu_o", [2, NL], f32, kind="ExternalOutput")
    value_o = nc.dram_tensor("value_o", [1, 1], f32, kind="ExternalOutput")

    with tile.TileContext(nc) as tc, ExitStack() as ctx:
        const = ctx.enter_context(tc.tile_pool(name="const", bufs=1))
        work = ctx.enter_context(tc.tile_pool(name="work", bufs=2))
        tpool = ctx.enter_context(tc.tile_pool(name="tpool", bufs=12))
        ppool = ctx.enter_context(tc.tile_pool(name="ppool", bufs=8))
        ps = ctx.enter_context(tc.tile_pool(name="ps", bufs=8, space="PSUM"))
        dram = ctx.enter_context(tc.tile_pool(name="dram", bufs=1, space="DRAM"))

        PSF = [128, 512]  # one PSUM bank of f32

        def psum():
            t = ps.tile(PSF, f32, tag="ps", name="pst")
            return t

        # ---- load constants to SBUF ----
        sb = {}
        for name, t in di.items():
            if name == "adjT_sh":
                continue  # read directly from DRAM by the mask-build DMAs
            s = const.tile(list(t.shape), t.dtype, tag=f"c_{name}", name=f"s_{name}")
            nc.sync.dma_start(s[:], t[:])
            sb[name] = s

        # ---- per-core mask tiles: mask_t[jc][j, (i,h)] = (adj[i, j]-1)*1e9 ----
        mask_t = []
        for jc in range(8):
            m = const.tile([128, 512], f32, tag=f"mask{jc}", name=f"mask{jc}")
            adjc = tpool.tile([128, NL], f32, tag="adjc", name=f"adjc{jc}")
            nc.sync.dma_start(adjc[:], di["adjT_sh"][jc * 128:(jc + 1) * 128, :])
            nc.vector.tensor_scalar(adjc[:], adjc[:], -1.0, 1e9,
                                    op0=ALU.add, op1=ALU.mult)
            for h in range(HEADS):
                if h % 2 == 0:
                    nc.vector.tensor_copy(m[:, h:512:HEADS], adjc[:])
                else:
                    nc.scalar.copy(m[:, h:512:HEADS], adjc[:])
            mask_t.append(m)

        # ---- gamma/beta broadcast tiles ----
        gb, bb = [], []
        for l in range(NLAYERS):
            gp = psum()
            nc.tensor.matmul(gp[:, :HID], sb["ones1"][:], sb[f"gam{l}"][:],
                             start=True, stop=True)
            g = const.tile([HID, HID], f32, tag=f"gb{l}", name=f"gb{l}")
            nc.vector.tensor_copy(g[:], gp[:, :HID])
            gb.append(g)
            bp_ = psum()
            nc.tensor.matmul(bp_[:, :HID], sb["ones1"][:], sb[f"bet{l}"][:],
                             start=True, stop=True)
            b = const.tile([HID, HID], f32, tag=f"bb{l}", name=f"bb{l}")
            nc.scalar.copy(b[:], bp_[:, :HID])
            bb.append(b)

        # ---- layer 0 input projection: hT (features on partitions) ----
        hT = work.tile([HID, N], f32, tag="hT", name="hT0")
        for half in range(2):
            pp = psum()
            nc.tensor.matmul(pp[:], sb["Wp"][:], sb["nfT"][:, half * 512:(half + 1) * 512],
                             start=True, stop=True)
            nc.scalar.activation(hT[:, half * 512:(half + 1) * 512], pp[:],
                                 AF.Relu, bias=sb["bp_c"][:, 0:1])
        pl = psum()
        nc.tensor.matmul(pl[:, :NL], sb["Wp"][:], sb["nfT_sh"][:], start=True, stop=True)
        hlT = work.tile([HID, NL], f32, tag="hlT", name="hlT0")
        nc.scalar.activation(hlT[:], pl[:, :NL], AF.Relu, bias=sb["bp_c"][:, 0:1])
        pr = psum()
        nc.tensor.transpose(pr[:, :NL], hlT[:], sb["eye"][:])
        h_loc = work.tile([NL, HID], f32, tag="h_loc", name="h_loc0")
        nc.vector.tensor_copy(h_loc[:], pr[:, :NL])

        # ================= GAT layers =================
        loop_ctx = tc.For_i(0, reps, 1) if reps > 1 else contextlib.nullcontext()
        with loop_ctx:
         for l in range(NLAYERS):
            # ---- stage A: projections ----
            pw = psum()
            nc.tensor.matmul(pw[:, :NL], sb[f"Wl{l}"][:], hlT[:], start=True, stop=True)
            wlhT = work.tile([HID, NL], f32, tag="wlhT", name=f"wlhT{l}")
            nc.vector.tensor_copy(wlhT[:], pw[:, :NL])

            wrhTb = work.tile([HID, N], f16, tag="wrhTb", name=f"wrhTb{l}")
            for half in range(2):
                pp = psum()
                for q in range(4):
                    jc = half * 4 + q
                    nc.tensor.matmul(pp[:, q * 128:(q + 1) * 128],
                                     sb[f"Wr{l}"][:],
                                     hT[:, jc * 128:(jc + 1) * 128],
                                     start=True, stop=True)
                    d = wrhTb[:, jc * 128:(jc + 1) * 128]
                    s_ = pp[:, q * 128:(q + 1) * 128]
                    if jc % 2 == 0:
                        nc.vector.tensor_copy(d, s_)
                    else:
                        nc.scalar.copy(d, s_)

            v_b = work.tile([128, N], bf16, tag="v_b", name=f"v_b{l}")
            for jc in range(8):
                pp = psum()
                nc.tensor.matmul(pp[:, :HID], hT[:, jc * 128:(jc + 1) * 128],
                                 sb[f"Wv{l}"][:], start=True, stop=True)
                nc.vector.tensor_copy(v_b[:, jc * 128:(jc + 1) * 128], pp[:, :HID])

            # ---- stage B: attention logits, transposed [j, (i,h)] ----
            ep = []
            a02b = sb[f"a02_{l}"][:].unsqueeze(1).broadcast_to([HID, 128, HEADS])
            for jc in range(8):
                e = ps.tile(PSF, f32, tag="ps", name=f"ep{l}_{jc}")
                nc.tensor.matmul(e[:].rearrange("p (i h) -> p i h", h=HEADS),
                                 wrhTb[:, jc * 128:(jc + 1) * 128], a02b,
                                 start=True, stop=False)
                ep.append(e)
            for i in range(NL):
                T_i = tpool.tile([HID, N], f16, tag="T", name=f"T{l}_{i}")
                if _ENG[i] == 'D':
                    nc.vector.tensor_scalar(T_i[:], wrhTb[:], wlhT[:, i:i + 1], 0.0,
                                            op0=ALU.add, op1=ALU.max)
                else:
                    nc.scalar.activation(T_i[:], wrhTb[:], AF.Relu,
                                         bias=wlhT[:, i:i + 1])
                for jc in range(8):
                    nc.tensor.matmul(ep[jc][:, 4 * i:4 * i + 4],
                                     T_i[:, jc * 128:(jc + 1) * 128],
                                     sb[f"a08_{l}"][:],
                                     start=False, stop=(i == NL - 1))

            # ---- stage C: mask, exp, row sums ----
            p_b = []
            for jc in range(8):
                nc.vector.scalar_tensor_tensor(ep[jc][:], ep[jc][:], 0.0, mask_t[jc][:],
                                               op0=ALU.add, op1=ALU.add)
                p = ppool.tile([128, 512], bf16, tag="p", name=f"p{l}_{jc}")
                nc.scalar.activation(p[:], ep[jc][:], AF.Exp)
                p_b.append(p)
            sp = psum()
            for jc in range(8):
                nc.tensor.matmul(sp[0:1, :], sb["onesb"][:], p_b[jc][:],
                                 start=(jc == 0), stop=(jc == 7))
            rs_row = work.tile([1, 512], f32, tag="rs_row", name=f"rs_row{l}")
            nc.vector.reciprocal(rs_row[:], sp[0:1, :])
            rs_dram = dram.tile([1, 512], f32, name=f"rs_d{l}")
            nc.sync.dma_start(rs_dram[:], rs_row[:])
            rs_t = work.tile([NL, HEADS], f32, tag="rs_t", name=f"rs_t{l}")
            nc.sync.dma_start(rs_t[:], rs_dram[:].rearrange("o (i h) -> (o i) h", h=HEADS))

            # ---- stage D: aggregate values ----
            agT = [ps.tile(PSF, f32, tag="ps", name=f"agT{l}_{h}") for h in range(HEADS)]
            for jc in range(8):
                for h in range(HEADS):
                    nc.tensor.matmul(agT[h][:, :HID], v_b[:, jc * 128:(jc + 1) * 128],
                                     p_b[jc][:, h:512:4],
                                     start=(jc == 0), stop=(jc == 7))
            aggT = work.tile([HID, NL], f32, tag="aggT", name=f"aggT{l}")
            for h in range(HEADS):
                nc.vector.tensor_copy(aggT[32 * h:32 * h + 32, :],
                                      agT[h][32 * h:32 * h + 32, :HID])
            pa = psum()
            nc.tensor.transpose(pa[:, :NL], aggT[:], sb["eye"][:])
            # normalize rows by 1/s per (i, head-block)
            for h in range(HEADS):
                nc.vector.tensor_scalar(pa[:, 32 * h:32 * h + 32],
                                        pa[:, 32 * h:32 * h + 32],
                                        rs_t[:, h:h + 1], None, op0=ALU.mult)

            # ---- stage E: LayerNorm + residual ----
            mu = work.tile([NL, 1], f32, tag="mu", name=f"mu{l}")
            nc.vector.tensor_reduce(mu[:], pa[:, :HID], AX.X, ALU.add)
            nc.vector.tensor_scalar(mu[:], mu[:], 1.0 / HID, None, op0=ALU.mult)
            xc = work.tile([NL, HID], f32, tag="xc", name=f"xc{l}")
            nc.vector.tensor_scalar(xc[:], pa[:, :HID], mu[:, 0:1], None, op0=ALU.subtract)
            sq = tpool.tile([NL, HID], bf16, tag="sq", name=f"sq{l}")
            var = work.tile([NL, 1], f32, tag="var", name=f"var{l}")
            nc.scalar.activation(sq[:], xc[:], AF.Square, accum_out=var[:, 0:1])
            nc.vector.tensor_scalar(var[:], var[:], 1.0 / HID, EPS, op0=ALU.mult, op1=ALU.add)
            std = work.tile([NL, 1], f32, tag="std", name=f"std{l}")
            nc.scalar.activation(std[:], var[:], AF.Sqrt)
            rstd = work.tile([NL, 1], f32, tag="rstd", name=f"rstd{l}")
            nc.vector.reciprocal(rstd[:], std[:])
            t1 = work.tile([NL, HID], f32, tag="t1", name=f"t1_{l}")
            nc.vector.scalar_tensor_tensor(t1[:], xc[:], rstd[:, 0:1], gb[l][:],
                                           op0=ALU.mult, op1=ALU.mult)
            nc.vector.scalar_tensor_tensor(t1[:], t1[:], 0.0, bb[l][:],
                                           op0=ALU.add, op1=ALU.add)
            h_new = work.tile([NL, HID], f32, tag="h_loc", name=f"h_loc{l + 1}")
            nc.vector.scalar_tensor_tensor(h_new[:], t1[:], 0.0, h_loc[:],
                                           op0=ALU.max, op1=ALU.add)
            h_loc = h_new
            ph = psum()
            nc.tensor.transpose(ph[:, :NL], h_loc[:], sb["eye"][:])
            hlT = work.tile([HID, NL], f32, tag="hlT", name=f"hlT{l + 1}")
            nc.scalar.copy(hlT[:], ph[:, :NL])

            # ---- all-gather h for next layer ----
            if l < NLAYERS - 1:
                gin = dram.tile([NL, HID], f32, name=f"gin{l}")
                hfull = dram.tile([N, HID], f32, name=f"hfull{l}")
                nc.sync.dma_start(gin[:], h_loc[:])
                if collectives:
                    nc.gpsimd.collective_compute(
                        "AllGather", ALU.bypass,
                        ins=[gin.opt()], outs=[hfull.opt()],
                        replica_groups=[list(range(NCORES))],
                    )
                else:
                    for c in range(NCORES):
                        nc.sync.dma_start(hfull[c * NL:(c + 1) * NL, :], gin[:])
                hT = work.tile([HID, N], f32, tag="hT", name=f"hT{l + 1}")
                for jc in range(8):
                    hrow = tpool.tile([128, HID], f32, tag="hrow", name=f"hrow{l}_{jc}")
                    nc.sync.dma_start(hrow[:], hfull[jc * 128:(jc + 1) * 128, :])
                    pt = psum()
                    nc.tensor.transpose(pt[:, :128], hrow[:], sb["eye"][:])
                    if jc % 2 == 0:
                        nc.vector.tensor_copy(hT[:, jc * 128:(jc + 1) * 128], pt[:, :128])
                    else:
                        nc.scalar.copy(hT[:, jc * 128:(jc + 1) * 128], pt[:, :128])

        # ================= output heads =================
        pz = psum()
        nc.tensor.matmul(pz[:64, :NL], sb["Wn1"][:], hlT[:], start=True, stop=True)
        z1 = work.tile([64, NL], f32, tag="z1", name="z1n")
        nc.scalar.activation(z1[:], pz[:64, :NL], AF.Relu, bias=sb["bn1_c"][:, 0:1])
        pz2 = psum()
        nc.tensor.matmul(pz2[0:1, :NL], sb["Wn2"][:], z1[:], start=True, stop=True)
        lg = work.tile([1, NL], f32, tag="lg", name="lg")
        nc.vector.tensor_scalar(lg[:], pz2[0:1, :NL], sb["bn2_c"][0:1, 0:1], None, op0=ALU.add)
        nc.sync.dma_start(logits_o[:], lg[:])

        pd = psum()
        nc.tensor.matmul(pd[:64, :NL], sb["Wd1"][:], hlT[:], start=True, stop=True)
        zd = work.tile([64, NL], f32, tag="zd", name="zd")
        nc.scalar.activation(zd[:], pd[:64, :NL], AF.Relu, bias=sb["bd1_c"][:, 0:1])
        pd2 = psum()
        nc.tensor.matmul(pd2[0:2, :NL], sb["Wd2"][:], zd[:], start=True, stop=True)
        dm = work.tile([2, NL], f32, tag="dm", name="dm")
        nc.vector.tensor_scalar(dm[:], pd2[0:2, :NL], sb["bd2_c"][0:2, 0:1], None, op0=ALU.add)
        nc.sync.dma_start(dmu_o[:], dm[:])

        # value head: mean-pool via all-reduce of partial sums
        ppool_c = work.tile([HID, 1], f32, tag="ppool_c", name="ppool_c")
        nc.vector.tensor_reduce(ppool_c[:], hlT[:], AX.X, ALU.add)
        nc.vector.tensor_scalar(ppool_c[:], ppool_c[:], 1.0 / N, None, op0=ALU.mult)
        pin = dram.tile([1, HID], f32, name="pin")
        pout = dram.tile([1, HID], f32, name="pout")
        nc.sync.dma_start(pin[:].rearrange("a b -> b a"), ppool_c[:])
        if collectives:
            nc.gpsimd.collective_compute(
                "AllReduce", ALU.add,
                ins=[pin.opt()], outs=[pout.opt()],
                replica_groups=[list(range(NCORES))],
            )
        else:
            nc.sync.dma_start(pout[:], pin[:])
        pooled = work.tile([HID, 1], f32, tag="pooled", name="pooled")
        nc.sync.dma_start(pooled[:], pout[:].rearrange("a b -> b a"))
        pv = psum()
        nc.tensor.matmul(pv[:64, 0:1], sb["Wu1"][:], pooled[:], start=True, stop=True)
        v1 = work.tile([64, 1], f32, tag="v1", name="v1")
        nc.scalar.activation(v1[:], pv[:64, 0:1], AF.Relu, bias=sb["bv1_c"][:, 0:1])
        pv2 = psum()
        nc.tensor.matmul(pv2[0:1, 0:1], sb["Wu2"][:], v1[:], start=True, stop=True)
        vv = work.tile([1, 1], f32, tag="vv", name="vv")
        nc.vector.tensor_scalar(vv[:], pv2[0:1, 0:1], sb["bv2_c"][0:1, 0:1], None, op0=ALU.add)
        nc.sync.dma_start(value_o[:], vv[:])

    nc.finalize()
    return nc


def prep_inputs(inputs):
    """Shard/layout the reference inputs into 8 per-core input maps."""
    nf = np.asarray(inputs["node_features"], np.float32)
    adj = np.asarray(inputs["adj"], np.float32)
    att = np.asarray(inputs["att"], np.float32)

    def col(x, p):
        return np.ascontiguousarray(np.asarray(x, np.float32).reshape(p, 1))

    nfT = np.ascontiguousarray(nf.T)
    shared = {
        "nfT": nfT,
        "Wp": np.asarray(inputs["Wp"], np.float32),
        "ones1": np.ones((1, HID), np.float32),
        "onesb": np.ones((HID, 1), ml_dtypes.bfloat16),
        "eye": np.eye(HID, dtype=np.float32),
        "bp_c": col(inputs["bp"], HID),
        "bn1_c": col(inputs["bn1"], 64), "bn2_c": col(inputs["bn2"], 1),
        "bd1_c": col(inputs["bd1"], 64), "bd2_c": col(inputs["bd2"], 2),
        "bv1_c": col(inputs["bv1"], 64), "bv2_c": col(inputs["bv2"], 1),
        "Wn1": np.asarray(inputs["Wn1"], np.float32),
        "Wn2": np.asarray(inputs["Wn2"], np.float32),
        "Wd1": np.asarray(inputs["Wd1"], np.float32),
        "Wd2": np.asarray(inputs["Wd2"], np.float32),
        "Wu1": np.asarray(inputs["Wv1"], np.float32),
        "Wu2": np.asarray(inputs["Wv2"], np.float32),
    }
    for l in range(NLAYERS):
        shared[f"Wl{l}"] = np.asarray(inputs["Wl"][l], np.float32)
        shared[f"Wr{l}"] = np.asarray(inputs["Wr"][l], np.float32)
        shared[f"Wv{l}"] = np.asarray(inputs["Wv"][l], np.float32)
        aM = np.kron(np.eye(HEADS, dtype=np.float32), att[l][:, None])  # [128, 4]
        shared[f"a02_{l}"] = (0.2 * aM).astype(np.float16)
        shared[f"a08_{l}"] = (0.8 * aM).astype(np.float16)
        shared[f"gam{l}"] = np.asarray(inputs["gamma"][l], np.float32).reshape(1, HID)
        shared[f"bet{l}"] = np.asarray(inputs["beta"][l], np.float32).reshape(1, HID)

    in_maps = []
    for c in range(NCORES):
        m = dict(shared)
        m["nfT_sh"] = np.ascontiguousarray(nfT[:, c * NL:(c + 1) * NL])
        m["adjT_sh"] = np.ascontiguousarray(adj[c * NL:(c + 1) * NL, :].T)
        in_maps.append(m)
    return in_maps


_CACHE = {}


def kernel(**inputs):
    from concourse.bass_utils import run_bass_kernel_spmd
    import os

    if "nc" not in _CACHE:
        _CACHE["nc"] = build()
    nc = _CACHE["nc"]

    in_maps = prep_inputs(inputs)
    trace = bool(int(os.environ.get("KERNEL_TRACE", "0")))
    res = run_bass_kernel_spmd(nc, in_maps, core_ids=list(range(NCORES)),
                               trace=trace)
    _CACHE["last_results"] = res

    outs = res.results
    node_logits = np.concatenate([outs[c]["logits_o"][0] for c in range(NCORES)])
    delta_mu = np.concatenate([outs[c]["dmu_o"].T for c in range(NCORES)], axis=0)
    value = np.float32(outs[0]["value_o"][0, 0])
    return (node_logits.astype(np.float32),
            delta_mu.astype(np.float32),
            value)


# revision 15
# speedup vs baseline: 1.0226x; 1.0226x over previous
"""GATv2 policy network on 8 Trainium2 NeuronCores.

Row-parallel attention: each core owns 128 source rows i of the N=1024
graph. The O(N^2*H*D) pair tensor is never materialized in HBM: per
local row i we build T_i = relu(wrhT + wlh_i) (the relu part of
leaky_relu = 0.2*x + 0.8*relu(x)) in SBUF and contract it against the
attention vector on the PE with T_i as the stationary operand, so the
logits land already transposed [j, (i,h)] filling all 8 PSUM banks.
The 0.2-linear term in wrh is pre-accumulated into PSUM by a broadcast
matmul; the wlh linear term is constant over j and cancels in softmax.
h is all-gathered between layers; a [1,128] all-reduce feeds the value
head's mean-pool.
"""

import numpy as np
import ml_dtypes
from contextlib import ExitStack

import concourse.bass as bass
import concourse.tile as tile
from concourse import bacc, mybir

f32 = mybir.dt.float32
f16 = mybir.dt.float16
bf16 = mybir.dt.bfloat16
f32r = mybir.dt.float32r
AF = mybir.ActivationFunctionType
ALU = mybir.AluOpType
AX = mybir.AxisListType

N = 1024
HID = 128
NLAYERS = 3
HEADS = 4
HD = 32
NCORES = 8
NL = N // NCORES  # 128 local rows
EPS = 1e-5

# stage-B T-build engine split across DVE / ScalarE / GpSimd,
# shares ~ inverse of per-inst cost (327 / 1146 / 1517 ns)
_PAT = ['D', 'D', 'A', 'D', 'D', 'D', 'D', 'A', 'D', 'D', 'D', 'A', 'D', 'D', 'D']
_ENG = [_PAT[i % 15] for i in range(NL)]


def build(enable_asserts=False, collectives=True, reps=1):
    import contextlib
    nc = bacc.Bacc("TRN2", target_bir_lowering=False, debug=False,
                   enable_asserts=enable_asserts, num_devices=NCORES)

    di = {}

    def inp(name, shape, dt=f32):
        di[name] = nc.dram_tensor(name, list(shape), dt, kind="ExternalInput")
        return di[name]

    inp("nfT", [3, N]); inp("nfT_sh", [3, NL]); inp("adjT_sh", [N, NL])
    inp("Wp", [3, HID])
    for l in range(NLAYERS):
        inp(f"Wl{l}", [HID, HID]); inp(f"Wr{l}", [HID, HID]); inp(f"Wv{l}", [HID, HID])
        inp(f"a02_{l}", [HID, HEADS], f16); inp(f"a08_{l}", [HID, HEADS], f16)
        inp(f"gam{l}", [1, HID]); inp(f"bet{l}", [1, HID])
    inp("ones1", [1, HID]); inp("onesb", [HID, 1], bf16); inp("eye", [HID, HID])
    inp("bp_c", [HID, 1]); inp("bn1_c", [64, 1]); inp("bn2_c", [1, 1])
    inp("bd1_c", [64, 1]); inp("bd2_c", [2, 1]); inp("bv1_c", [64, 1]); inp("bv2_c", [1, 1])
    inp("Wn1", [HID, 64]); inp("Wn2", [64, 1])
    inp("Wd1", [HID, 64]); inp("Wd2", [64, 2])
    inp("Wu1", [HID, 64]); inp("Wu2", [64, 1])

    logits_o = nc.dram_tensor("logits_o", [1, NL], f32, kind="ExternalOutput")
    dmu_o = nc.dram_tensor("dmu_o", [2, NL], f32, kind="ExternalOutput")
    value_o = nc.dram_tensor("value_o", [1, 1], f32, kind="ExternalOutput")

    with tile.TileContext(nc) as tc, ExitStack() as ctx:
        const = ctx.enter_context(tc.tile_pool(name="const", bufs=1))
        work = ctx.enter_context(tc.tile_pool(name="work", bufs=2))
        tpool = ctx.enter_context(tc.tile_pool(name="tpool", bufs=12))
        ppool = ctx.enter_context(tc.tile_pool(name="ppool", bufs=8))
        ps = ctx.enter_context(tc.tile_pool(name="ps", bufs=8, space="PSUM"))
        dram = ctx.enter_context(tc.tile_pool(name="dram", bufs=1, space="DRAM"))

        PSF = [128, 512]  # one PSUM bank of f32

        def psum():
            t = ps.tile(PSF, f32, tag="ps", name="pst")
            return t

        # ---- load constants to SBUF ----
        sb = {}
        for name, t in di.items():
            if name == "adjT_sh":
                continue  # read directly from DRAM by the mask-build DMAs
            s = const.tile(list(t.shape), t.dtype, tag=f"c_{name}", name=f"s_{name}")
            nc.sync.dma_start(s[:], t[:])
            sb[name] = s

        # ---- per-core mask tiles: mask_t[jc][j, (i,h)] = (adj[i, j]-1)*1e9 ----
        adj4 = []
        for jc in range(8):
            m = const.tile([128, 512], bf16, tag=f"adj4_{jc}", name=f"adj4_{jc}")
            adjc = tpool.tile([128, NL], f32, tag="adjc", name=f"adjc{jc}")
            nc.sync.dma_start(adjc[:], di["adjT_sh"][jc * 128:(jc + 1) * 128, :])
            for h in range(HEADS):
                if h % 2 == 0:
                    nc.vector.tensor_copy(m[:, h:512:HEADS], adjc[:])
                else:
                    nc.scalar.copy(m[:, h:512:HEADS], adjc[:])
            adj4.append(m)

        # ---- gamma/beta broadcast tiles ----
        gb, bb = [], []
        for l in range(NLAYERS):
            gp = psum()
            nc.tensor.matmul(gp[:, :HID], sb["ones1"][:], sb[f"gam{l}"][:],
                             start=True, stop=True)
            g = const.tile([HID, HID], f32, tag=f"gb{l}", name=f"gb{l}")
            nc.vector.tensor_copy(g[:], gp[:, :HID])
            gb.append(g)
            bp_ = psum()
            nc.tensor.matmul(bp_[:, :HID], sb["ones1"][:], sb[f"bet{l}"][:],
                             start=True, stop=True)
            b = const.tile([HID, HID], f32, tag=f"bb{l}", name=f"bb{l}")
            nc.scalar.copy(b[:], bp_[:, :HID])
            bb.append(b)

        # ---- layer 0 input projection: hT (features on partitions) ----
        hT = work.tile([HID, N], f32, tag="hT", name="hT0")
        for half in range(2):
            pp = psum()
            nc.tensor.matmul(pp[:], sb["Wp"][:], sb["nfT"][:, half * 512:(half + 1) * 512],
                             start=True, stop=True)
            nc.scalar.activation(hT[:, half * 512:(half + 1) * 512], pp[:],
                                 AF.Relu, bias=sb["bp_c"][:, 0:1])
        pl = psum()
        nc.tensor.matmul(pl[:, :NL], sb["Wp"][:], sb["nfT_sh"][:], start=True, stop=True)
        hlT = work.tile([HID, NL], f32, tag="hlT", name="hlT0")
        nc.scalar.activation(hlT[:], pl[:, :NL], AF.Relu, bias=sb["bp_c"][:, 0:1])
        pr = psum()
        nc.tensor.transpose(pr[:, :NL], hlT[:], sb["eye"][:])
        h_loc = work.tile([NL, HID], f32, tag="h_loc", name="h_loc0")
        nc.vector.tensor_copy(h_loc[:], pr[:, :NL])

        # ================= GAT layers =================
        loop_ctx = tc.For_i(0, reps, 1) if reps > 1 else contextlib.nullcontext()
        with loop_ctx:
         for l in range(NLAYERS):
            # ---- stage A: projections ----
            pw = psum()
            nc.tensor.matmul(pw[:, :NL], sb[f"Wl{l}"][:], hlT[:], start=True, stop=True)
            wlhT = work.tile([HID, NL], f32, tag="wlhT", name=f"wlhT{l}")
            nc.vector.tensor_copy(wlhT[:], pw[:, :NL])

            wrhTb = work.tile([HID, N], f16, tag="wrhTb", name=f"wrhTb{l}")
            for half in range(2):
                pp = psum()
                for q in range(4):
                    jc = half * 4 + q
                    nc.tensor.matmul(pp[:, q * 128:(q + 1) * 128],
                                     sb[f"Wr{l}"][:],
                                     hT[:, jc * 128:(jc + 1) * 128],
                                     start=True, stop=True)
                    d = wrhTb[:, jc * 128:(jc + 1) * 128]
                    s_ = pp[:, q * 128:(q + 1) * 128]
                    if jc % 2 == 0:
                        nc.vector.tensor_copy(d, s_)
                    else:
                        nc.scalar.copy(d, s_)

            v_b = work.tile([128, N], bf16, tag="v_b", name=f"v_b{l}")
            for jc in range(8):
                pp = psum()
                nc.tensor.matmul(pp[:, :HID], hT[:, jc * 128:(jc + 1) * 128],
                                 sb[f"Wv{l}"][:], start=True, stop=True)
                nc.vector.tensor_copy(v_b[:, jc * 128:(jc + 1) * 128], pp[:, :HID])

            # ---- stage B: attention logits, transposed [j, (i,h)] ----
            ep = []
            a02b = sb[f"a02_{l}"][:].unsqueeze(1).broadcast_to([HID, 128, HEADS])
            for jc in range(8):
                e = ps.tile(PSF, f32, tag="ps", name=f"ep{l}_{jc}")
                nc.tensor.matmul(e[:].rearrange("p (i h) -> p i h", h=HEADS),
                                 wrhTb[:, jc * 128:(jc + 1) * 128], a02b,
                                 start=True, stop=False)
                ep.append(e)
            for i in range(NL):
                T_i = tpool.tile([HID, N], f16, tag="T", name=f"T{l}_{i}")
                if _ENG[i] == 'D':
                    nc.vector.tensor_scalar(T_i[:], wrhTb[:], wlhT[:, i:i + 1], 0.0,
                                            op0=ALU.add, op1=ALU.max)
                else:
                    nc.scalar.activation(T_i[:], wrhTb[:], AF.Relu,
                                         bias=wlhT[:, i:i + 1])
                for jc in range(8):
                    nc.tensor.matmul(ep[jc][:, 4 * i:4 * i + 4],
                                     T_i[:, jc * 128:(jc + 1) * 128],
                                     sb[f"a08_{l}"][:],
                                     start=False, stop=(i == NL - 1))

            # ---- stage C: mask, exp, row sums ----
            p_b = []
            for jc in range(8):
                p = ppool.tile([128, 512], bf16, tag="p", name=f"p{l}_{jc}")
                nc.scalar.activation(p[:], ep[jc][:], AF.Exp)
                nc.vector.tensor_tensor(p[:], p[:], adj4[jc][:], op=ALU.mult)
                p_b.append(p)
            sp = psum()
            for jc in range(8):
                nc.tensor.matmul(sp[0:1, :], sb["onesb"][:], p_b[jc][:],
                                 start=(jc == 0), stop=(jc == 7))
            rs_row = work.tile([1, 512], f32, tag="rs_row", name=f"rs_row{l}")
            nc.vector.reciprocal(rs_row[:], sp[0:1, :])
            rs_dram = dram.tile([1, 512], f32, name=f"rs_d{l}")
            nc.sync.dma_start(rs_dram[:], rs_row[:])
            rs_t = work.tile([NL, HEADS], f32, tag="rs_t", name=f"rs_t{l}")
            nc.sync.dma_start(rs_t[:], rs_dram[:].rearrange("o (i h) -> (o i) h", h=HEADS))

            # ---- stage D: aggregate values ----
            agT = [ps.tile(PSF, f32, tag="ps", name=f"agT{l}_{h}") for h in range(HEADS)]
            for jc in range(8):
                for h in range(HEADS):
                    nc.tensor.matmul(agT[h][:, :HID], v_b[:, jc * 128:(jc + 1) * 128],
                                     p_b[jc][:, h:512:4],
                                     start=(jc == 0), stop=(jc == 7))
            aggT = work.tile([HID, NL], f32, tag="aggT", name=f"aggT{l}")
            for h in range(HEADS):
                nc.vector.tensor_copy(aggT[32 * h:32 * h + 32, :],
                                      agT[h][32 * h:32 * h + 32, :HID])
            pa = psum()
            nc.tensor.transpose(pa[:, :NL], aggT[:], sb["eye"][:])
            # normalize rows by 1/s per (i, head-block)
            for h in range(HEADS):
                nc.vector.tensor_scalar(pa[:, 32 * h:32 * h + 32],
                                        pa[:, 32 * h:32 * h + 32],
                                        rs_t[:, h:h + 1], None, op0=ALU.mult)

            # ---- stage E: LayerNorm + residual ----
            mu = work.tile([NL, 1], f32, tag="mu", name=f"mu{l}")
            nc.vector.tensor_reduce(mu[:], pa[:, :HID], AX.X, ALU.add)
            nc.vector.tensor_scalar(mu[:], mu[:], 1.0 / HID, None, op0=ALU.mult)
            xc = work.tile([NL, HID], f32, tag="xc", name=f"xc{l}")
            nc.vector.tensor_scalar(xc[:], pa[:, :HID], mu[:, 0:1], None, op0=ALU.subtract)
            sq = tpool.tile([NL, HID], bf16, tag="sq", name=f"sq{l}")
            var = work.tile([NL, 1], f32, tag="var", name=f"var{l}")
            nc.scalar.activation(sq[:], xc[:], AF.Square, accum_out=var[:, 0:1])
            nc.vector.tensor_scalar(var[:], var[:], 1.0 / HID, EPS, op0=ALU.mult, op1=ALU.add)
            std = work.tile([NL, 1], f32, tag="std", name=f"std{l}")
            nc.scalar.activation(std[:], var[:], AF.Sqrt)
            rstd = work.tile([NL, 1], f32, tag="rstd", name=f"rstd{l}")
            nc.vector.reciprocal(rstd[:], std[:])
            t1 = work.tile([NL, HID], f32, tag="t1", name=f"t1_{l}")
            nc.vector.scalar_tensor_tensor(t1[:], xc[:], rstd[:, 0:1], gb[l][:],
                                           op0=ALU.mult, op1=ALU.mult)
            nc.vector.scalar_tensor_tensor(t1[:], t1[:], 0.0, bb[l][:],
                                           op0=ALU.add, op1=ALU.add)
            h_new = work.tile([NL, HID], f32, tag="h_loc", name=f"h_loc{l + 1}")
            nc.vector.scalar_tensor_tensor(h_new[:], t1[:], 0.0, h_loc[:],
                                           op0=ALU.max, op1=ALU.add)
            h_loc = h_new
            ph = psum()
            nc.tensor.transpose(ph[:, :NL], h_loc[:], sb["eye"][:])
            hlT = work.tile([HID, NL], f32, tag="hlT", name=f"hlT{l + 1}")
            nc.scalar.copy(hlT[:], ph[:, :NL])

            # ---- all-gather hT (feature-major) for next layer ----
            if l < NLAYERS - 1:
                gin = dram.tile([HID, NL], f32, name=f"gin{l}")
                hfullT = dram.tile([N, NL], f32, name=f"hfullT{l}")
                nc.sync.dma_start(gin[:], hlT[:])
                if collectives:
                    nc.gpsimd.collective_compute(
                        "AllGather", ALU.bypass,
                        ins=[gin.opt()], outs=[hfullT.opt()],
                        replica_groups=[list(range(NCORES))],
                    )
                else:
                    for c in range(NCORES):
                        nc.sync.dma_start(hfullT[c * HID:(c + 1) * HID, :], gin[:])
                hT = work.tile([HID, N], f32, tag="hT", name=f"hT{l + 1}")
                for c in range(NCORES):
                    nc.sync.dma_start(hT[:, c * NL:(c + 1) * NL],
                                      hfullT[c * HID:(c + 1) * HID, :])

        # ================= output heads =================
        pz = psum()
        nc.tensor.matmul(pz[:64, :NL], sb["Wn1"][:], hlT[:], start=True, stop=True)
        z1 = work.tile([64, NL], f32, tag="z1", name="z1n")
        nc.scalar.activation(z1[:], pz[:64, :NL], AF.Relu, bias=sb["bn1_c"][:, 0:1])
        pz2 = psum()
        nc.tensor.matmul(pz2[0:1, :NL], sb["Wn2"][:], z1[:], start=True, stop=True)
        lg = work.tile([1, NL], f32, tag="lg", name="lg")
        nc.vector.tensor_scalar(lg[:], pz2[0:1, :NL], sb["bn2_c"][0:1, 0:1], None, op0=ALU.add)
        nc.sync.dma_start(logits_o[:], lg[:])

        pd = psum()
        nc.tensor.matmul(pd[:64, :NL], sb["Wd1"][:], hlT[:], start=True, stop=True)
        zd = work.tile([64, NL], f32, tag="zd", name="zd")
        nc.scalar.activation(zd[:], pd[:64, :NL], AF.Relu, bias=sb["bd1_c"][:, 0:1])
        pd2 = psum()
        nc.tensor.matmul(pd2[0:2, :NL], sb["Wd2"][:], zd[:], start=True, stop=True)
        dm = work.tile([2, NL], f32, tag="dm", name="dm")
        nc.vector.tensor_scalar(dm[:], pd2[0:2, :NL], sb["bd2_c"][0:2, 0:1], None, op0=ALU.add)
        nc.sync.dma_start(dmu_o[:], dm[:])

        # value head: mean-pool via all-reduce of partial sums
        ppool_c = work.tile([HID, 1], f32, tag="ppool_c", name="ppool_c")
        nc.vector.tensor_reduce(ppool_c[:], hlT[:], AX.X, ALU.add)
        nc.vector.tensor_scalar(ppool_c[:], ppool_c[:], 1.0 / N, None, op0=ALU.mult)
        pin = dram.tile([1, HID], f32, name="pin")
        pout = dram.tile([1, HID], f32, name="pout")
        nc.sync.dma_start(pin[:].rearrange("a b -> b a"), ppool_c[:])
        if collectives:
            nc.gpsimd.collective_compute(
                "AllReduce", ALU.add,
                ins=[pin.opt()], outs=[pout.opt()],
                replica_groups=[list(range(NCORES))],
            )
        else:
            nc.sync.dma_start(pout[:], pin[:])
        pooled = work.tile([HID, 1], f32, tag="pooled", name="pooled")
        nc.sync.dma_start(pooled[:], pout[:].rearrange("a b -> b a"))
        pv = psum()
        nc.tensor.matmul(pv[:64, 0:1], sb["Wu1"][:], pooled[:], start=True, stop=True)
        v1 = work.tile([64, 1], f32, tag="v1", name="v1")
        nc.scalar.activation(v1[:], pv[:64, 0:1], AF.Relu, bias=sb["bv1_c"][:, 0:1])
        pv2 = psum()
        nc.tensor.matmul(pv2[0:1, 0:1], sb["Wu2"][:], v1[:], start=True, stop=True)
        vv = work.tile([1, 1], f32, tag="vv", name="vv")
        nc.vector.tensor_scalar(vv[:], pv2[0:1, 0:1], sb["bv2_c"][0:1, 0:1], None, op0=ALU.add)
        nc.sync.dma_start(value_o[:], vv[:])

    nc.finalize()
    return nc


def prep_inputs(inputs):
    """Shard/layout the reference inputs into 8 per-core input maps."""
    nf = np.asarray(inputs["node_features"], np.float32)
    adj = np.asarray(inputs["adj"], np.float32)
    att = np.asarray(inputs["att"], np.float32)

    def col(x, p):
        return np.ascontiguousarray(np.asarray(x, np.float32).reshape(p, 1))

    nfT = np.ascontiguousarray(nf.T)
    shared = {
        "nfT": nfT,
        "Wp": np.asarray(inputs["Wp"], np.float32),
        "ones1": np.ones((1, HID), np.float32),
        "onesb": np.ones((HID, 1), ml_dtypes.bfloat16),
        "eye": np.eye(HID, dtype=np.float32),
        "bp_c": col(inputs["bp"], HID),
        "bn1_c": col(inputs["bn1"], 64), "bn2_c": col(inputs["bn2"], 1),
        "bd1_c": col(inputs["bd1"], 64), "bd2_c": col(inputs["bd2"], 2),
        "bv1_c": col(inputs["bv1"], 64), "bv2_c": col(inputs["bv2"], 1),
        "Wn1": np.asarray(inputs["Wn1"], np.float32),
        "Wn2": np.asarray(inputs["Wn2"], np.float32),
        "Wd1": np.asarray(inputs["Wd1"], np.float32),
        "Wd2": np.asarray(inputs["Wd2"], np.float32),
        "Wu1": np.asarray(inputs["Wv1"], np.float32),
        "Wu2": np.asarray(inputs["Wv2"], np.float32),
    }
    for l in range(NLAYERS):
        shared[f"Wl{l}"] = np.asarray(inputs["Wl"][l], np.float32)
        shared[f"Wr{l}"] = np.asarray(inputs["Wr"][l], np.float32)
        shared[f"Wv{l}"] = np.asarray(inputs["Wv"][l], np.float32)
        aM = np.kron(np.eye(HEADS, dtype=np.float32), att[l][:, None])  # [128, 4]
        shared[f"a02_{l}"] = (0.2 * aM).astype(np.float16)
        shared[f"a08_{l}"] = (0.8 * aM).astype(np.float16)
        shared[f"gam{l}"] = np.asarray(inputs["gamma"][l], np.float32).reshape(1, HID)
        shared[f"bet{l}"] = np.asarray(inputs["beta"][l], np.float32).reshape(1, HID)

    in_maps = []
    for c in range(NCORES):
        m = dict(shared)
        m["nfT_sh"] = np.ascontiguousarray(nfT[:, c * NL:(c + 1) * NL])
        m["adjT_sh"] = np.ascontiguousarray(adj[c * NL:(c + 1) * NL, :].T)
        in_maps.append(m)
    return in_maps


_CACHE = {}


def kernel(**inputs):
    from concourse.bass_utils import run_bass_kernel_spmd
    import os

    if "nc" not in _CACHE:
        _CACHE["nc"] = build()
    nc = _CACHE["nc"]

    in_maps = prep_inputs(inputs)
    trace = bool(int(os.environ.get("KERNEL_TRACE", "0")))
    res = run_bass_kernel_spmd(nc, in_maps, core_ids=list(range(NCORES)),
                               trace=trace)
    _CACHE["last_results"] = res

    outs = res.results
    node_logits = np.concatenate([outs[c]["logits_o"][0] for c in range(NCORES)])
    delta_mu = np.concatenate([outs[c]["dmu_o"].T for c in range(NCORES)], axis=0)
    value = np.float32(outs[0]["value_o"][0, 0])
    return (node_logits.astype(np.float32),
            delta_mu.astype(np.float32),
            value)
